# revision 28
# baseline (speedup 1.0000x reference)
"""Trainium2 Bass kernel for the ARqGPSFull autoregressive wavefunction.

Math: out[b] = sum_{s,m} ctx[b,s,m] * I_{x[b,s]}[s,m]; in logs the masked
product is affine in x, so with centered x' = x-0.5 and the observed-state
selection folded into the diagonal (see _host_pack):
  T[b,s,m] = exp(Sre) * (cos(Sim) + i sin(Sim)),  S = x'^T D + C
Each core owns 8 of the 64 m-values -> 512 (s,m) columns.  D ships as a
bf16 hi+lo pair (two accumulating matmuls, 66+64 contraction rows) so S is
fp32-exact; the constants ride rows 64/65 of the hi block, split hi/lo.

v4 structure (vs the 20.0us baseline):
- ONE act table set for the whole kernel: pwp set 22 (exp_and_friends)
  holds exp AND sin2pi, killing the 1283ns mid-kernel table switch.
  sin2pi shares opcode 4 with sin; walrus knows the BIR name "Sin2pi"
  (to_json_bytes shim), inputs pre-scaled by 1/(2pi).  Probe: exact to
  6e-8 on |x| <= 0.5 including the edges.
- No range reduction: half-angle identities with a direct cosine.
    sq = sin(Sim/2)                       (|Sim/2| < pi, in domain)
    cq = cos(Sim/2) = sin2pi(Sim/(4pi) + 1/4)   (bias rides the ACT op)
    cos(Sim) = 1 - 2 sq^2,  sin(Sim) = 2 sq cq
  ACT is the only PSUM reader (cross-engine same-bank PSUM reads
  serialize), and Pool touches no 512-wide f32 tensors (it runs them
  2-3x slower than DVE and contends for SBUF).
- Product-sums collapse onto the common factor u = pe*sq:
    o1 = sum(2 u cq) = Tim      o2 = sum(2 u sq) = aE - Tre
  so the tail is 3 ACT ops (exp+2 sins) + 3 DVE ops (u, scrR, scrI) +
  a [B,1] Pool subtract + the DVE block-transpose.
- 4 matmuls of N=512 (re-hi 66, re-lo 64, im-hi 66, im-lo 64); exp waits
  only the two re matmuls.
- 3 input DMAs grouped by arrival need and spread across the three
  DMA-capable engines so no single sequencer serializes the issues:
  [XT|RH] -> SP, [RL] -> ACT, [IMH|IML] -> Pool; all hoisted pre-barrier
  post-compile together with the act-table load; the preamble Pool DMA
  drain is deleted.  Output: one DMA on ACT + one on SP (same-queue
  equal-shape DMAs get mis-merged by the compiler).
"""

import sys

for _p in ("/opt/trn_rl_repo", "/root/.axon_site/_ro/trn_rl_repo"):
    if _p not in sys.path:
        sys.path.append(_p)

import math
import numpy as np
import ml_dtypes

N_CORES = 8
B = 128        # batch
L = 64         # n_sites
M = 64         # GPS support dim
NM = M // N_CORES   # m-values per core
NBLK = L * NM  # 512 (s,m) columns per core
PI = math.pi

_BF16 = ml_dtypes.bfloat16

_built = None

# bf16 tensor: [XT | RH | IMH]; fp8(e5m2) tensor: [RL | IML].
# The lo-halves of D fit e5m2's exponent range with no scaling, and the
# hi/lo constant rows stay on rows 64/65 of the bf16 hi blocks.
C_XT = 0
C_RH = B
C_IMH = C_RH + NBLK
C_TOT1 = C_IMH + NBLK
F_RL = 0
F_IML = NBLK
C_TOT2 = 2 * NBLK


def _build():
    import concourse.bacc as bacc
    import concourse.mybir as mybir
    from concourse import tile

    f32 = mybir.dt.float32
    bf16 = mybir.dt.bfloat16
    AF = mybir.ActivationFunctionType
    ALU = mybir.AluOpType

    fp8 = mybir.dt.float8e5
    nc = bacc.Bacc()
    rhs_d = nc.dram_tensor("rhs", [66, C_TOT1], bf16, kind="ExternalInput")
    rhs2_d = nc.dram_tensor("rhs2", [64, C_TOT2], fp8, kind="ExternalInput")
    out_d = nc.dram_tensor("out", [4, 64], f32, kind="ExternalOutput")

    with tile.TileContext(nc) as tc:
        with (
            tc.tile_pool(name="pc", bufs=1) as pc,
            tc.tile_pool(name="psum", bufs=1, space="PSUM") as psum,
        ):
            rhs = pc.tile([66, C_TOT1], bf16, tag="rhs")
            rhs2 = pc.tile([64, C_TOT2], fp8, tag="rhs2")
            # four input DMAs; per-queue FIFO order follows descriptor
            # write time, so issue order = arrival order: [XT|RH] on SP
            # first, [RL] on Pool, [IMH] as SP's second issue, [IML] as
            # Pool's second.  Arrivals match consumption order and exp
            # waits only the two re matmuls.
            nc.sync.dma_start(rhs[:, C_XT:C_IMH], rhs_d[:, C_XT:C_IMH])
            nc.gpsimd.dma_start(rhs2[:, F_RL:F_IML], rhs2_d[:, F_RL:F_IML])
            nc.sync.dma_start(rhs[:, C_IMH:C_TOT1], rhs_d[:, C_IMH:C_TOT1])
            nc.gpsimd.dma_start(rhs2[:, F_IML:C_TOT2],
                                rhs2_d[:, F_IML:C_TOT2])
            xt66 = rhs[:, C_XT:C_XT + B]
            xt64 = rhs[0:64, C_XT:C_XT + B]

            o = pc.tile([B, 64], f32, tag="o")
            nc.gpsimd.memset(o[:], 0.0)
            quarter = pc.tile([B, 1], f32, tag="quarter")
            nc.gpsimd.memset(quarter[:], 0.25)
            import os as _os
            _warm = _os.environ.get("WARMUP", "0") == "1"
            if _warm:
                # PE pstate warmup: two throwaway matmuls on never-written
                # tiles (garbage in, garbage out -- Sr is starte'd over by
                # the real matmuls).  Hoisted to the preamble post-compile
                # so they run during the input-DMA wait.
                dw = pc.tile([66, B], bf16, tag="dw")
                dm = pc.tile([66, NBLK], bf16, tag="dm")
                nc.vector.memset(dw[:], 0.0)
                nc.vector.memset(dm[:], 0.0)

            Sr = psum.tile([B, NBLK], f32, tag="Sr")
            Si = psum.tile([B, NBLK], f32, tag="Si")
            pe = pc.tile([B, NBLK], f32, tag="pe")
            if _warm:
                nc.tensor.matmul(Sr[:], dw[:], dm[:], start=True, stop=True)
                nc.tensor.matmul(Sr[:], dw[:], dm[:], start=True, stop=True)
            nc.tensor.matmul(Sr[:], xt66, rhs[:, C_RH:C_RH + NBLK],
                             start=True, stop=False)
            nc.tensor.matmul(Sr[:], xt64, rhs2[:, F_RL:F_RL + NBLK],
                             start=False, stop=True)
            # pe = exp(Sre), aE = sum(pe) free from the ACT accumulator
            nc.scalar.activation(pe[:], Sr[:], AF.Exp, accum_out=o[:, 0:1])
            nc.tensor.matmul(Si[:], xt66, rhs[:, C_IMH:C_IMH + NBLK],
                             start=True, stop=False)
            nc.tensor.matmul(Si[:], xt64, rhs2[:, F_IML:F_IML + NBLK],
                             start=False, stop=True)

            # sq = sin(Sim/2), cq = cos(Sim/2): ACT only, no range reduction
            sq = pc.tile([B, NBLK], f32, tag="sq")
            cq = pc.tile([B, NBLK], f32, tag="cq")
            nc.scalar.activation(sq[:], Si[:], AF.Sin, scale=0.25 / PI)
            nc.scalar.activation(cq[:], Si[:], AF.Sin, scale=0.25 / PI,
                                 bias=quarter[:])

            # u = pe*sq; o1 = sum(2 u cq) = Tim; o2 = sum(2 u sq) = aE - Tre
            u = pc.tile([B, NBLK], f32, tag="u")
            nc.vector.tensor_mul(u[:], pe[:], sq[:])
            scrR = pc.tile([B, NBLK], f32, tag="scrR")
            nc.vector.scalar_tensor_tensor(
                scrR[:], u[:], 2.0, sq[:], op0=ALU.mult, op1=ALU.mult,
                accum_out=o[:, 33:34])
            scrI = pc.tile([B, NBLK], f32, tag="scrI")
            nc.vector.scalar_tensor_tensor(
                scrI[:], u[:], 2.0, cq[:], op0=ALU.mult, op1=ALU.mult,
                accum_out=o[:, 1:2])
            # oRe = aE - sum(2 u sq) via ACT Identity(-o2 + aE_bias);
            # column 33 so that after the 32x32 block transpose both
            # quantities live on the same rows 32k+1 (im in cols 0:32,
            # re in cols 32:64) and ONE output DMA covers both.
            nc.scalar.activation(o[:, 33:34], o[:, 33:34], AF.Identity,
                                 bias=o[:, 0:1], scale=-1.0)

            # block-transpose so the output DMA rows are contiguous:
            # tr[32k+c, 32j+p] = o[32k+p, 32j+c]
            tr = pc.tile([B, 64], f32, tag="tr")
            nc.vector.transpose(tr[:], o[:])
            nc.sync.dma_start(out_d[:], tr[1:99:32, :])

    nc.compile()

    import os
    mybir_ET = mybir.EngineType

    # --- single act table: set 22 (exp + sin2pi), delete other loads ----
    loads = [(b, ins) for b in nc.main_func.blocks
             for ins in b.instructions
             if type(ins).__name__ == "InstLoadActFuncSet"]
    assert loads, "no act table load found"
    loads[0][1].act_func_set_id = 22
    for b, ins in loads[1:]:
        si = ins.sync_info
        assert si is None or (not si.on_wait and not si.on_update)
        b.instructions.remove(ins)

    # Sin -> Sin2pi in the serialized BIR (walrus-native name)
    _orig_json = nc.to_json_bytes
    nc.to_json_bytes = (
        lambda: _orig_json().replace(b'"func":"Sin"', b'"func":"Sin2pi"'))

    # --- drop the post-ISA barrier round in the teardown block ----------
    # Block 2 runs: SP waits (incl. output-DMA completion), a full
    # 5-engine gather/release barrier, Pool Drain + InstISA (the NEFF
    # completion marker), then a SECOND identical barrier round.  After
    # round 1 the barrier sems are balanced (release back to 0) and all
    # engines are drained, so round 2 is pure epilogue cost; delete it.
    if os.environ.get("KEEP_ROUND2") != "1":
        b2 = nc.main_func.blocks[2]
        isa_idx = next((i for i, ins in enumerate(b2.instructions)
                        if type(ins).__name__ == "InstISA"), None)
        if isa_idx is not None:
            del b2.instructions[isa_idx + 1:]
    # The Pool SWDGE drains in the teardown cost ~1.5us each; DMA
    # completion is already guaranteed by SP's DMASW0 semaphore wait,
    # so drop them (mirrors the preamble drain deletion).
    if os.environ.get("KEEP_B2_DRAIN") != "1":
        b2 = nc.main_func.blocks[2]
        for ins in list(b2.instructions):
            if (type(ins).__name__ == "InstDrain"
                    and ins.engine == mybir_ET.Pool):
                si = ins.sync_info
                assert si is None or (not si.on_wait and not si.on_update)
                b2.instructions.remove(ins)
    # SP's teardown waits serially poll input-DMA and engine semaphores
    # that are all transitively implied by the output-DMA completion
    # (DMAHW2): the output DMA waited the transpose, which waited the
    # whole compute graph, which waited every input DMA.  Keep only the
    # wait that mentions DMAHW2.
    if os.environ.get("KEEP_SP_WAITS") != "1":
        b2 = nc.main_func.blocks[2]
        for ins in list(b2.instructions):
            if (type(ins).__name__ == "InstEventSemaphore"
                    and ins.engine == mybir_ET.SP):
                si = ins.sync_info
                if si and si.on_wait and not any(
                        w.ant_name.startswith("DMAHW2")
                        for w in si.on_wait) and not si.on_update:
                    b2.instructions.remove(ins)



    if os.environ.get("NO_HOIST") == "1":
        return nc

    # --- hoist wait-free input DMAs + act table load into the preamble --
    b0, b1 = nc.main_func.blocks[0], nc.main_func.blocks[1]
    hoist = []
    pe_hoist_open = True
    for ins in list(b1.instructions):
        nm = type(ins).__name__
        if nm == "InstDMACopy" and ins.engine in (mybir_ET.Pool,
                                                  mybir_ET.SP,
                                                  mybir_ET.Activation):
            si = ins.sync_info
            if si is not None and si.on_wait:
                continue  # output DMAs wait on results
            hoist.append(ins)
            b1.instructions.remove(ins)
        elif nm == "InstLoadActFuncSet":
            si = ins.sync_info
            assert si is None or (not si.on_wait and not si.on_update)
            hoist.append(ins)
            b1.instructions.remove(ins)
        elif nm == "InstMemset" and ins.engine == mybir_ET.DVE:
            hoist.append(ins)
            b1.instructions.remove(ins)
        elif (nm in ("InstLdweights", "InstMatmult")
                and ins.engine == mybir_ET.PE and pe_hoist_open):
            # leading PE instructions = the pstate warmup mms; they may
            # wait the DVE memset sem (fine across blocks) but the first
            # DMA-gated instruction is the real matmul chain
            si = ins.sync_info
            if si is not None and any(w.ant_name.startswith("DMA")
                                      for w in si.on_wait):
                pe_hoist_open = False
                continue
            hoist.append(ins)
            b1.instructions.remove(ins)
    for ins in reversed(hoist):
        first = next((i for i, x in enumerate(b0.instructions)
                      if x.engine == ins.engine), len(b0.instructions))
        b0.instructions.insert(first, ins)
    if os.environ.get("KEEP_DRAIN") != "1":
        for ins in list(b0.instructions):
            if (type(ins).__name__ == "InstDrain"
                    and ins.engine == mybir_ET.Pool):
                b0.instructions.remove(ins)
    return nc


def _host_pack(inputs, params_context, inputs_param):
    x = np.asarray(inputs).astype(np.float64)          # (B, L) in {0,1}
    P = np.asarray(params_context)                     # (s, d, m, j) complex
    I = np.asarray(inputs_param)                       # (s, d, m) complex

    mask = (np.arange(L)[None, :] < np.maximum(np.arange(L), 1)[:, None])
    Lp = np.log(P)
    D = (Lp[:, 1] - Lp[:, 0]) * mask[:, None, :]       # (s, m, j)
    C = (Lp[:, 0] * mask[:, None, :]).sum(-1)          # (s, m)
    I0 = I[:, 0]
    I1 = I[:, 1]
    A0 = np.log(np.abs(I0))
    dA = np.log(np.abs(I1)) - A0
    wrap = lambda t: np.angle(np.exp(1j * t))
    ph0 = np.angle(I0)
    dPh = wrap(np.angle(I1) - ph0)
    eye = np.eye(L)[:, None, :]                        # (s, 1, j)
    Dre = D.real + eye * dA[:, :, None]                # (s, m, j)
    Dim = D.imag + eye * dPh[:, :, None]
    CA = C.real + A0 + 0.5 * Dre.sum(-1)               # x-centering shift
    PH = wrap(C.imag + ph0 + 0.5 * Dim.sum(-1))

    xt = np.concatenate([(x - 0.5).T, np.ones((2, B))], 0)  # (66, B)
    _F8 = ml_dtypes.float8_e5m2
    rhs_list = []
    for k in range(N_CORES):
        msl = slice(k * NM, (k + 1) * NM)
        full = np.zeros((66, C_TOT1), np.float64)
        full2 = np.zeros((64, C_TOT2), np.float64)
        full[:, C_XT:C_XT + B] = xt
        for Dp, const, chi, flo in (
                (Dre, CA, C_RH, F_RL),
                (Dim, PH, C_IMH, F_IML)):
            Dc = Dp[:, msl, :].transpose(2, 0, 1).reshape(L, NBLK)  # (j, sm)
            Dhi = Dc.astype(_BF16).astype(np.float64)
            Dlo = Dc - Dhi
            cc = const[:, msl].reshape(NBLK)
            hi = cc.astype(_BF16).astype(np.float64)
            lo = cc - hi
            full[0:64, chi:chi + NBLK] = Dhi
            full2[:, flo:flo + NBLK] = Dlo
            full[64, chi:chi + NBLK] = hi
            full[65, chi:chi + NBLK] = lo
        rhs_list.append((full.astype(_BF16), full2.astype(_F8)))
    return rhs_list


def kernel(inputs, params_context, inputs_param):
    global _built
    from concourse.bass_utils import run_bass_kernel_spmd

    if _built is None:
        _built = _build()
    nc = _built

    rhs_list = _host_pack(inputs, params_context, inputs_param)
    in_maps = [{"rhs": rhs_list[k][0], "rhs2": rhs_list[k][1]}
               for k in range(N_CORES)]
    res = run_bass_kernel_spmd(nc, in_maps, list(range(N_CORES)))

    re = np.zeros(B, np.float64)
    im = np.zeros(B, np.float64)
    for k in range(N_CORES):
        q = np.asarray(res.results[k]["out"], np.float64)  # (4, 64)
        im += q[:, 0:32].reshape(B)   # o1 = +Tim in this formulation
        re += q[:, 32:64].reshape(B)
    return (re + 1j * np.angle(np.exp(1j * im))).astype(np.complex128)


# revision 29
# speedup vs baseline: 1.0443x; 1.0443x over previous
"""Trainium2 Bass kernel for the ARqGPSFull autoregressive wavefunction.

Math: out[b] = sum_{s,m} ctx[b,s,m] * I_{x[b,s]}[s,m]; in logs the masked
product is affine in x, so with centered x' = x-0.5 and the observed-state
selection folded into the diagonal (see _host_pack):
  T[b,s,m] = exp(Sre) * (cos(Sim) + i sin(Sim)),  S = x'^T D + C
Each core owns 8 of the 64 m-values -> 512 (s,m) columns.  D ships as a
bf16 hi+lo pair (two accumulating matmuls, 66+64 contraction rows) so S is
fp32-exact; the constants ride rows 64/65 of the hi block, split hi/lo.

v4 structure (vs the 20.0us baseline):
- ONE act table set for the whole kernel: pwp set 22 (exp_and_friends)
  holds exp AND sin2pi, killing the 1283ns mid-kernel table switch.
  sin2pi shares opcode 4 with sin; walrus knows the BIR name "Sin2pi"
  (to_json_bytes shim), inputs pre-scaled by 1/(2pi).  Probe: exact to
  6e-8 on |x| <= 0.5 including the edges.
- No range reduction: half-angle identities with a direct cosine.
    sq = sin(Sim/2)                       (|Sim/2| < pi, in domain)
    cq = cos(Sim/2) = sin2pi(Sim/(4pi) + 1/4)   (bias rides the ACT op)
    cos(Sim) = 1 - 2 sq^2,  sin(Sim) = 2 sq cq
  ACT is the only PSUM reader (cross-engine same-bank PSUM reads
  serialize), and Pool touches no 512-wide f32 tensors (it runs them
  2-3x slower than DVE and contends for SBUF).
- Product-sums collapse onto the common factor u = pe*sq:
    o1 = sum(2 u cq) = Tim      o2 = sum(2 u sq) = aE - Tre
  so the tail is 3 ACT ops (exp+2 sins) + 3 DVE ops (u, scrR, scrI) +
  a [B,1] Pool subtract + the DVE block-transpose.
- 4 matmuls of N=512 (re-hi 66, re-lo 64, im-hi 66, im-lo 64); exp waits
  only the two re matmuls.
- 3 input DMAs grouped by arrival need and spread across the three
  DMA-capable engines so no single sequencer serializes the issues:
  [XT|RH] -> SP, [RL] -> ACT, [IMH|IML] -> Pool; all hoisted pre-barrier
  post-compile together with the act-table load; the preamble Pool DMA
  drain is deleted.  Output: one DMA on ACT + one on SP (same-queue
  equal-shape DMAs get mis-merged by the compiler).
"""

import sys

for _p in ("/opt/trn_rl_repo", "/root/.axon_site/_ro/trn_rl_repo"):
    if _p not in sys.path:
        sys.path.append(_p)

import math
import numpy as np
import ml_dtypes

N_CORES = 8
B = 128        # batch
L = 64         # n_sites
M = 64         # GPS support dim
NM = M // N_CORES   # m-values per core
NBLK = L * NM  # 512 (s,m) columns per core
PI = math.pi

_BF16 = ml_dtypes.bfloat16

_built = None

# bf16 tensor: [XT | RH | IMH]; fp8(e5m2) tensor: [RL | IML].
# The lo-halves of D fit e5m2's exponent range with no scaling, and the
# hi/lo constant rows stay on rows 64/65 of the bf16 hi blocks.
C_XT = 0
C_RH = B
C_IMH = C_RH + NBLK
C_TOT1 = C_IMH + NBLK
F_RL = 0
F_IML = NBLK
C_TOT2 = 2 * NBLK


def _build():
    import concourse.bacc as bacc
    import concourse.mybir as mybir
    from concourse import tile

    f32 = mybir.dt.float32
    bf16 = mybir.dt.bfloat16
    AF = mybir.ActivationFunctionType
    ALU = mybir.AluOpType

    fp8 = mybir.dt.float8e5
    nc = bacc.Bacc()
    rhs_d = nc.dram_tensor("rhs", [66, C_TOT1], bf16, kind="ExternalInput")
    rhs2_d = nc.dram_tensor("rhs2", [64, C_TOT2], fp8, kind="ExternalInput")
    out_d = nc.dram_tensor("out", [4, 64], f32, kind="ExternalOutput")

    with tile.TileContext(nc) as tc:
        with (
            tc.tile_pool(name="pc", bufs=1) as pc,
            tc.tile_pool(name="psum", bufs=1, space="PSUM") as psum,
        ):
            rhs = pc.tile([66, C_TOT1], bf16, tag="rhs")
            rhs2 = pc.tile([64, C_TOT2], fp8, tag="rhs2")
            # three input DMAs; per-queue FIFO order follows descriptor
            # write time, so issue order = arrival order: [XT|RH] on SP
            # first, the whole fp8 [RL|IML] on Pool (512B rows, lands
            # second), [IMH] as SP's second issue (im-hi runs third, so
            # the last arrival is fine).  exp waits only the two re
            # matmuls; the PSUM hi->lo accumulation order is enforced by
            # the start/stop chain.
            nc.sync.dma_start(rhs[:, C_XT:C_IMH], rhs_d[:, C_XT:C_IMH])
            nc.gpsimd.dma_start(rhs2[:], rhs2_d[:])
            nc.sync.dma_start(rhs[:, C_IMH:C_TOT1], rhs_d[:, C_IMH:C_TOT1])
            xt66 = rhs[:, C_XT:C_XT + B]
            xt64 = rhs[0:64, C_XT:C_XT + B]

            o = pc.tile([B, 64], f32, tag="o")
            nc.gpsimd.memset(o[:], 0.0)
            quarter = pc.tile([B, 1], f32, tag="quarter")
            nc.gpsimd.memset(quarter[:], 0.25)
            import os as _os
            _warm = _os.environ.get("WARMUP", "0") == "1"
            if _warm:
                # PE pstate warmup: two throwaway matmuls on never-written
                # tiles (garbage in, garbage out -- Sr is starte'd over by
                # the real matmuls).  Hoisted to the preamble post-compile
                # so they run during the input-DMA wait.
                dw = pc.tile([66, B], bf16, tag="dw")
                dm = pc.tile([66, NBLK], bf16, tag="dm")
                nc.vector.memset(dw[:], 0.0)
                nc.vector.memset(dm[:], 0.0)

            Sr = psum.tile([B, NBLK], f32, tag="Sr")
            Si = psum.tile([B, NBLK], f32, tag="Si")
            pe = pc.tile([B, NBLK], f32, tag="pe")
            if _warm:
                nc.tensor.matmul(Sr[:], dw[:], dm[:], start=True, stop=True)
                nc.tensor.matmul(Sr[:], dw[:], dm[:], start=True, stop=True)
            nc.tensor.matmul(Sr[:], xt66, rhs[:, C_RH:C_RH + NBLK],
                             start=True, stop=False)
            nc.tensor.matmul(Sr[:], xt64, rhs2[:, F_RL:F_RL + NBLK],
                             start=False, stop=True)
            # pe = exp(Sre), aE = sum(pe) free from the ACT accumulator
            nc.scalar.activation(pe[:], Sr[:], AF.Exp, accum_out=o[:, 0:1])
            nc.tensor.matmul(Si[:], xt66, rhs[:, C_IMH:C_IMH + NBLK],
                             start=True, stop=False)
            nc.tensor.matmul(Si[:], xt64, rhs2[:, F_IML:F_IML + NBLK],
                             start=False, stop=True)

            # sq = sin(Sim/2), cq = cos(Sim/2): ACT only, no range reduction
            sq = pc.tile([B, NBLK], f32, tag="sq")
            cq = pc.tile([B, NBLK], f32, tag="cq")
            nc.scalar.activation(sq[:], Si[:], AF.Sin, scale=0.25 / PI)
            nc.scalar.activation(cq[:], Si[:], AF.Sin, scale=0.25 / PI,
                                 bias=quarter[:])

            # u = pe*sq; o1 = sum(2 u cq) = Tim; o2 = sum(2 u sq) = aE - Tre
            u = pc.tile([B, NBLK], f32, tag="u")
            nc.vector.tensor_mul(u[:], pe[:], sq[:])
            scrR = pc.tile([B, NBLK], f32, tag="scrR")
            nc.vector.scalar_tensor_tensor(
                scrR[:], u[:], 2.0, sq[:], op0=ALU.mult, op1=ALU.mult,
                accum_out=o[:, 33:34])
            scrI = pc.tile([B, NBLK], f32, tag="scrI")
            nc.vector.scalar_tensor_tensor(
                scrI[:], u[:], 2.0, cq[:], op0=ALU.mult, op1=ALU.mult,
                accum_out=o[:, 1:2])
            # oRe = aE - sum(2 u sq) via ACT Identity(-o2 + aE_bias);
            # column 33 so that after the 32x32 block transpose both
            # quantities live on the same rows 32k+1 (im in cols 0:32,
            # re in cols 32:64) and ONE output DMA covers both.
            nc.scalar.activation(o[:, 33:34], o[:, 33:34], AF.Identity,
                                 bias=o[:, 0:1], scale=-1.0)

            # block-transpose so the output DMA rows are contiguous:
            # tr[32k+c, 32j+p] = o[32k+p, 32j+c]
            tr = pc.tile([B, 64], f32, tag="tr")
            nc.vector.transpose(tr[:], o[:])
            nc.sync.dma_start(out_d[:], tr[1:99:32, :])

    nc.compile()

    import os
    mybir_ET = mybir.EngineType

    # --- single act table: set 22 (exp + sin2pi), delete other loads ----
    loads = [(b, ins) for b in nc.main_func.blocks
             for ins in b.instructions
             if type(ins).__name__ == "InstLoadActFuncSet"]
    assert loads, "no act table load found"
    loads[0][1].act_func_set_id = 22
    for b, ins in loads[1:]:
        si = ins.sync_info
        assert si is None or (not si.on_wait and not si.on_update)
        b.instructions.remove(ins)

    # Sin -> Sin2pi in the serialized BIR (walrus-native name)
    _orig_json = nc.to_json_bytes
    nc.to_json_bytes = (
        lambda: _orig_json().replace(b'"func":"Sin"', b'"func":"Sin2pi"'))

    # --- drop the post-ISA barrier round in the teardown block ----------
    # Block 2 runs: SP waits (incl. output-DMA completion), a full
    # 5-engine gather/release barrier, Pool Drain + InstISA (the NEFF
    # completion marker), then a SECOND identical barrier round.  After
    # round 1 the barrier sems are balanced (release back to 0) and all
    # engines are drained, so round 2 is pure epilogue cost; delete it.
    if os.environ.get("KEEP_ROUND2") != "1":
        b2 = nc.main_func.blocks[2]
        isa_idx = next((i for i, ins in enumerate(b2.instructions)
                        if type(ins).__name__ == "InstISA"), None)
        if isa_idx is not None:
            del b2.instructions[isa_idx + 1:]
    # The Pool SWDGE drains in the teardown cost ~1.5us each; DMA
    # completion is already guaranteed by SP's DMASW0 semaphore wait,
    # so drop them (mirrors the preamble drain deletion).
    if os.environ.get("KEEP_B2_DRAIN") != "1":
        b2 = nc.main_func.blocks[2]
        for ins in list(b2.instructions):
            if (type(ins).__name__ == "InstDrain"
                    and ins.engine == mybir_ET.Pool):
                si = ins.sync_info
                assert si is None or (not si.on_wait and not si.on_update)
                b2.instructions.remove(ins)
    # SP's teardown waits serially poll input-DMA and engine semaphores
    # that are all transitively implied by the output-DMA completion
    # (DMAHW2): the output DMA waited the transpose, which waited the
    # whole compute graph, which waited every input DMA.  Keep only the
    # wait that mentions DMAHW2.
    if os.environ.get("KEEP_SP_WAITS") != "1":
        b2 = nc.main_func.blocks[2]
        for ins in list(b2.instructions):
            if (type(ins).__name__ == "InstEventSemaphore"
                    and ins.engine == mybir_ET.SP):
                si = ins.sync_info
                if si and si.on_wait and not any(
                        w.ant_name.startswith("DMAHW2")
                        for w in si.on_wait) and not si.on_update:
                    b2.instructions.remove(ins)



    if os.environ.get("NO_HOIST") == "1":
        return nc

    # --- hoist wait-free input DMAs + act table load into the preamble --
    b0, b1 = nc.main_func.blocks[0], nc.main_func.blocks[1]
    hoist = []
    pe_hoist_open = True
    for ins in list(b1.instructions):
        nm = type(ins).__name__
        if nm == "InstDMACopy" and ins.engine in (mybir_ET.Pool,
                                                  mybir_ET.SP,
                                                  mybir_ET.Activation):
            si = ins.sync_info
            if si is not None and si.on_wait:
                continue  # output DMAs wait on results
            hoist.append(ins)
            b1.instructions.remove(ins)
        elif nm == "InstLoadActFuncSet":
            si = ins.sync_info
            assert si is None or (not si.on_wait and not si.on_update)
            hoist.append(ins)
            b1.instructions.remove(ins)
        elif nm == "InstMemset" and ins.engine == mybir_ET.DVE:
            hoist.append(ins)
            b1.instructions.remove(ins)
        elif (nm in ("InstLdweights", "InstMatmult")
                and ins.engine == mybir_ET.PE and pe_hoist_open):
            # leading PE instructions = the pstate warmup mms; they may
            # wait the DVE memset sem (fine across blocks) but the first
            # DMA-gated instruction is the real matmul chain
            si = ins.sync_info
            if si is not None and any(w.ant_name.startswith("DMA")
                                      for w in si.on_wait):
                pe_hoist_open = False
                continue
            hoist.append(ins)
            b1.instructions.remove(ins)
    for ins in reversed(hoist):
        first = next((i for i, x in enumerate(b0.instructions)
                      if x.engine == ins.engine), len(b0.instructions))
        b0.instructions.insert(first, ins)
    if os.environ.get("KEEP_DRAIN") != "1":
        for ins in list(b0.instructions):
            if (type(ins).__name__ == "InstDrain"
                    and ins.engine == mybir_ET.Pool):
                b0.instructions.remove(ins)
    return nc


def _host_pack(inputs, params_context, inputs_param):
    x = np.asarray(inputs).astype(np.float64)          # (B, L) in {0,1}
    P = np.asarray(params_context)                     # (s, d, m, j) complex
    I = np.asarray(inputs_param)                       # (s, d, m) complex

    mask = (np.arange(L)[None, :] < np.maximum(np.arange(L), 1)[:, None])
    Lp = np.log(P)
    D = (Lp[:, 1] - Lp[:, 0]) * mask[:, None, :]       # (s, m, j)
    C = (Lp[:, 0] * mask[:, None, :]).sum(-1)          # (s, m)
    I0 = I[:, 0]
    I1 = I[:, 1]
    A0 = np.log(np.abs(I0))
    dA = np.log(np.abs(I1)) - A0
    wrap = lambda t: np.angle(np.exp(1j * t))
    ph0 = np.angle(I0)
    dPh = wrap(np.angle(I1) - ph0)
    eye = np.eye(L)[:, None, :]                        # (s, 1, j)
    Dre = D.real + eye * dA[:, :, None]                # (s, m, j)
    Dim = D.imag + eye * dPh[:, :, None]
    CA = C.real + A0 + 0.5 * Dre.sum(-1)               # x-centering shift
    PH = wrap(C.imag + ph0 + 0.5 * Dim.sum(-1))

    xt = np.concatenate([(x - 0.5).T, np.ones((2, B))], 0)  # (66, B)
    _F8 = ml_dtypes.float8_e5m2
    rhs_list = []
    for k in range(N_CORES):
        msl = slice(k * NM, (k + 1) * NM)
        full = np.zeros((66, C_TOT1), np.float64)
        full2 = np.zeros((64, C_TOT2), np.float64)
        full[:, C_XT:C_XT + B] = xt
        for Dp, const, chi, flo in (
                (Dre, CA, C_RH, F_RL),
                (Dim, PH, C_IMH, F_IML)):
            Dc = Dp[:, msl, :].transpose(2, 0, 1).reshape(L, NBLK)  # (j, sm)
            Dhi = Dc.astype(_BF16).astype(np.float64)
            Dlo = Dc - Dhi
            cc = const[:, msl].reshape(NBLK)
            hi = cc.astype(_BF16).astype(np.float64)
            lo = cc - hi
            full[0:64, chi:chi + NBLK] = Dhi
            full2[:, flo:flo + NBLK] = Dlo
            full[64, chi:chi + NBLK] = hi
            full[65, chi:chi + NBLK] = lo
        rhs_list.append((full.astype(_BF16), full2.astype(_F8)))
    return rhs_list


def kernel(inputs, params_context, inputs_param):
    global _built
    from concourse.bass_utils import run_bass_kernel_spmd

    if _built is None:
        _built = _build()
    nc = _built

    rhs_list = _host_pack(inputs, params_context, inputs_param)
    in_maps = [{"rhs": rhs_list[k][0], "rhs2": rhs_list[k][1]}
               for k in range(N_CORES)]
    res = run_bass_kernel_spmd(nc, in_maps, list(range(N_CORES)))

    re = np.zeros(B, np.float64)
    im = np.zeros(B, np.float64)
    for k in range(N_CORES):
        q = np.asarray(res.results[k]["out"], np.float64)  # (4, 64)
        im += q[:, 0:32].reshape(B)   # o1 = +Tim in this formulation
        re += q[:, 32:64].reshape(B)
    return (re + 1j * np.angle(np.exp(1j * im))).astype(np.complex128)
